# revision 12
# baseline (speedup 1.0000x reference)
"""Trainium2 Bass kernel for nn_DynamiSE (GNN message passing + RK4 ODE).

V3 layout: ALL 8 cores run BOTH ODEs (each core owns N/8=12500 nodes of
pos-state and neg-state). Per RK4 substep, each ODE does an 8-rank
AllGather with a Shared-space output table (fast path) and the pos
AllGather overlaps the neg compute. Phase-B gather indices/masks are
SBUF-resident (loaded once after phase A). Gather select runs pair-wise
(two 49-col segments per select pass) with merged window adds; the
select/accumulate datapath is fp16.

Entry point: kernel(**inputs) -> np.ndarray [100000, 32] float32.
"""

import numpy as np

H_ODE = 0.1


# ===================== host-side graph preprocessing =====================

def shard_meta(N, nshards, npos):
    per = N // nshards
    owner = np.minimum(np.arange(N) // per, nshards - 1)
    local = np.arange(N) - owner * per
    return owner, local, per


def degree_sort_positions(dst, owner_of, local_of, nshards, per, npos):
    N = len(owner_of)
    deg_all = np.bincount(dst, minlength=N)
    pos_of = np.empty(N, np.int64)
    for c in range(nshards):
        nodes = np.where(owner_of == c)[0]
        order = nodes[np.argsort(-deg_all[nodes], kind="stable")]
        pos_of[order] = np.arange(len(nodes))
    return pos_of, deg_all


def make_segments(cols_r_glob, seg_cols):
    """Split per-round column counts into gather segments."""
    segs = []
    pend_windows, pend_cols = [], 0

    def flush():
        nonlocal pend_windows, pend_cols
        if pend_windows:
            segs.append({"windows": pend_windows, "ncols": pend_cols})
            pend_windows, pend_cols = [], 0

    for r, cols in enumerate(cols_r_glob):
        if cols >= seg_cols // 2:
            flush()
            a = 0
            while a < cols:
                w = min(seg_cols, cols - a)
                segs.append({"windows": [(r, a, w, 0)], "ncols": w})
                a += w
        else:
            if pend_cols + cols > seg_cols:
                flush()
            pend_windows.append((r, 0, cols, pend_cols))
            pend_cols += cols
    flush()
    return segs


def wrap_indices(rows_stream):
    n = len(rows_stream)
    assert n % 128 == 0
    w = np.zeros((16, n // 16), np.int16)
    idxs = np.arange(n)
    w[idxs % 16, idxs // 16] = rows_stream.astype(np.int16)
    return np.tile(w, (8, 1))


def build_streams(src, dst, owner_of, pos_of, nshards, npos, row_of, sub_of,
                  seg_cols, coef=None):
    """Round/segment gather streams for one edge list over dst shards."""
    grp = npos // 4
    if coef is None:
        coef = np.ones(len(src), np.float32)
    shard_rounds = []
    for c in range(nshards):
        m = owner_of[dst] == c
        s_c = src[m]
        cf_c = coef[m]
        p_c = pos_of[dst[m]]
        order = np.lexsort((np.arange(len(s_c)), p_c))
        ps, ss, cs = p_c[order], s_c[order], cf_c[order]
        first = np.searchsorted(ps, ps)
        rank = np.arange(len(ps)) - first
        maxdeg = int(rank.max()) + 1 if len(rank) else 0
        rounds = []
        for r in range(maxdeg):
            sel = rank == r
            psr, ssr, csr = ps[sel], ss[sel], cs[sel]
            assert np.array_equal(psr, np.arange(len(psr)))
            rounds.append((len(psr), ssr, csr))
        shard_rounds.append(rounds)

    nr = max(len(r) for r in shard_rounds)
    cols_r = [max((len(sr[r][1]) if r < len(sr) else 0)
                  for sr in shard_rounds) for r in range(nr)]
    cols_r = [(n + 127) // 128 for n in cols_r]
    segs = make_segments(cols_r, seg_cols)
    totcols = sum(s["ncols"] for s in segs)

    per_shard = []
    for c in range(nshards):
        zero_row = c * grp + (grp - 1)
        rows_stream = np.full(totcols * 128, zero_row, np.int64)
        subs_stream = np.zeros(totcols * 128, np.int64)
        vals_stream = np.zeros(totcols * 128, np.float32)
        off = 0
        for s in segs:
            for (r, a, w, so) in s["windows"]:
                if r < len(shard_rounds[c]):
                    _, ssr, csr = shard_rounds[c][r]
                    lo, hi = a * 128, min((a + w) * 128, len(ssr))
                    if hi > lo:
                        dstslice = slice(off + so * 128,
                                         off + so * 128 + (hi - lo))
                        rows_stream[dstslice] = row_of[ssr[lo:hi]]
                        subs_stream[dstslice] = sub_of[ssr[lo:hi]]
                        vals_stream[dstslice] = csr[lo:hi]
            off += s["ncols"] * 128
        gidx = wrap_indices(rows_stream)
        masks = np.zeros((128, totcols, 4), np.float16)
        ii = np.arange(totcols * 128)
        masks[ii % 128, ii // 128, subs_stream] = vals_stream
        per_shard.append((gidx, masks))
    return segs, totcols, per_shard


def blockdiag4(W):
    fin, fout = W.shape
    assert fout == 32 and fin % 32 == 0
    tiles = []
    for k in range(fin // 32):
        t = np.zeros((128, 128), np.float32)
        for bi in range(4):
            t[32 * bi:32 * bi + 32, 32 * bi:32 * bi + 32] = \
                W[32 * k:32 * k + 32, :]
        tiles.append(t)
    return np.stack(tiles)


def build_all(inputs, seg_cols=33, nstep=10, seg_cols_b=33):
    N = inputs["H_t"].shape[0]
    NC = 8
    per8 = N // NC
    C8 = (per8 + 4 + 127) // 128
    POS8 = C8 * 128
    GRP8 = POS8 // 4

    edges_all = np.concatenate([inputs["A_pos_t"], inputs["A_neg_t"]],
                               axis=1).astype(np.int64)
    dp = inputs["dA_pos"].astype(np.int64)
    dn = inputs["dA_neg"].astype(np.int64)

    own8, loc8, _ = shard_meta(N, NC, POS8)

    posA, degA_all = degree_sort_positions(edges_all[1], own8, loc8, NC,
                                           per8, POS8)
    rowA = own8 * GRP8 + posA // 4
    subA = posA % 4
    posP, degP_all = degree_sort_positions(dp[1], own8, loc8, NC, per8, POS8)
    rowP = own8 * GRP8 + posP // 4
    subP = posP % 4
    posN, degN_all = degree_sort_positions(dn[1], own8, loc8, NC, per8, POS8)
    rowN = own8 * GRP8 + posN // 4
    subN = posN % 4

    dinvA_all = 1.0 / np.sqrt(1.0 + degA_all)
    dinvP_all = 1.0 / np.sqrt(1.0 + degP_all)
    dinvN_all = 1.0 / np.sqrt(1.0 + degN_all)
    coefA = (dinvA_all[edges_all[0]] * dinvA_all[edges_all[1]]).astype(
        np.float32)
    coefP = (dinvP_all[dp[0]] * dinvP_all[dp[1]]).astype(np.float32)
    coefN = (dinvN_all[dn[0]] * dinvN_all[dn[1]]).astype(np.float32)

    segA, TA, shA = build_streams(edges_all[0], edges_all[1], own8, posA,
                                  NC, POS8, rowA, subA, seg_cols, coef=coefA)
    segP, TP, shP = build_streams(dp[0], dp[1], own8, posP, NC, POS8, rowP,
                                  subP, seg_cols_b, coef=coefP)
    segN, TN, shN = build_streams(dn[0], dn[1], own8, posN, NC, POS8, rowN,
                                  subN, seg_cols_b, coef=coefN)

    node_at_posP = np.full((NC, POS8), -1, np.int64)
    node_at_posP[own8, posP] = np.arange(N)
    node_at_posN = np.full((NC, POS8), -1, np.int64)
    node_at_posN[own8, posN] = np.arange(N)

    segH = make_segments([C8], seg_cols)
    segC = make_segments([C8], seg_cols)

    wib = blockdiag4(np.asarray(inputs["W_init"], np.float32))
    wblk_p = blockdiag4(np.asarray(inputs["W_pos"], np.float32))[0] \
        .astype(np.float16)
    wblk_n = blockdiag4(np.asarray(inputs["W_neg"], np.float32))[0] \
        .astype(np.float16)
    cwb = blockdiag4(np.asarray(inputs["W_comb"], np.float32)) \
        .astype(np.float16)
    bi32 = np.tile(np.asarray(inputs["b_init"], np.float32), (128, 1))
    lng = np.tile(np.asarray(inputs["ln_g"], np.float32), (128, 1))
    lnb = np.tile(np.asarray(inputs["ln_b"], np.float32), (128, 1))
    cbstk = np.tile(np.asarray(inputs["b_comb"], np.float32), 4)[:, None]
    bstk_p = np.tile(np.asarray(inputs["b_pos"], np.float32), 4)[:, None]
    bstk_n = np.tile(np.asarray(inputs["b_neg"], np.float32), 4)[:, None]
    wt_p = np.tile(np.asarray(inputs["wt_pos"], np.float32), 4)[:, None]
    wt_n = np.tile(np.asarray(inputs["wt_neg"], np.float32), 4)[:, None]
    offs = np.array([0.0, 0.5, 0.5, 1.0]) * H_ODE
    tg = (np.arange(nstep)[:, None] * H_ODE + offs[None, :]).reshape(-1)
    tgrid = np.tile(tg.astype(np.float32), (128, 1))

    Ht = np.asarray(inputs["H_t"], np.float32)

    def pos_pack(vals, pos, c, fill=0.0):
        arr = np.full(POS8, fill, np.float32)
        nodes = np.where(own8 == c)[0]
        arr[pos[nodes]] = vals[nodes]
        return arr.reshape(C8, 128, 1).transpose(1, 0, 2).copy()

    in_maps = []
    for c in range(NC):
        nodesA = np.where(own8 == c)[0]
        ordA = nodesA[np.argsort(posA[nodesA])]
        ht_c = np.zeros((POS8, 128), np.float32)
        ht_c[:len(ordA)] = Ht[ordA]
        assert np.array_equal(posA[ordA], np.arange(len(ordA)))

        degA_c = pos_pack(dinvA_all * dinvA_all, posA, c)
        degP_c = pos_pack(dinvP_all * dinvP_all, posP, c)
        degN_c = pos_pack(dinvN_all * dinvN_all, posN, c)

        def h_redist(nap):
            rows = np.zeros(POS8, np.int64)
            subs = np.zeros(POS8, np.int64)
            valid = nap >= 0
            rows[valid] = rowA[nap[valid]]
            subs[valid] = subA[nap[valid]]
            gidx = wrap_indices(rows)
            masks = np.zeros((128, C8, 4), np.float16)
            ii = np.arange(POS8)
            m = np.zeros(POS8, np.float16)
            m[valid] = 1.0
            masks[ii % 128, ii // 128, subs] = m
            return gidx, masks

        gxHp_c, mHp_c = h_redist(node_at_posP[c])
        gxHn_c, mHn_c = h_redist(node_at_posN[c])

        tgt = np.arange(c * per8, (c + 1) * per8)

        def c_gather(row_of, sub_of):
            rows = np.zeros(POS8, np.int64)
            subs = np.zeros(POS8, np.int64)
            rows[:per8] = row_of[tgt]
            subs[:per8] = sub_of[tgt]
            gidx = wrap_indices(rows)
            masks = np.zeros((128, C8, 4), np.float16)
            ii = np.arange(POS8)
            m = np.zeros(POS8, np.float16)
            m[:per8] = 1.0
            masks[ii % 128, ii // 128, subs] = m
            return gidx, masks

        gxCp_c, mCp_c = c_gather(rowP, subP)
        gxCn_c, mCn_c = c_gather(rowN, subN)

        in_maps.append({
            "ht": ht_c, "degA": degA_c, "degP": degP_c, "degN": degN_c,
            "wib": wib, "wpos": wblk_p, "wneg": wblk_n, "cwb": cwb,
            "bi32": bi32, "bstkp": bstk_p, "bstkn": bstk_n, "cbstk": cbstk,
            "wtp": wt_p, "wtn": wt_n, "tgrid": tgrid, "lng": lng, "lnb": lnb,
            "gxA": shA[c][0], "mA": shA[c][1],
            "gxP": shP[c][0], "mP": shP[c][1],
            "gxN": shN[c][0], "mN": shN[c][1],
            "gxHp": gxHp_c, "mHp": mHp_c,
            "gxHn": gxHn_c, "mHn": mHn_c,
            "gxCp": gxCp_c, "mCp": mCp_c,
            "gxCn": gxCn_c, "mCn": mCn_c,
        })

    cfg = {"C8": C8, "TA": TA, "TP": TP, "TN": TN,
           "segA": segA, "segP": segP, "segN": segN,
           "segH": segH, "segC": segC, "NSTEP": nstep}
    meta = {"perA": per8, "cfg": cfg}
    return cfg, in_maps, meta


def assemble_output(results, perA, N):
    outs = [results[c]["o"][:perA] for c in range(8)]
    return np.concatenate(outs, axis=0)[:N]


# ============================ device program ============================

from concourse import bass, bacc, mybir
import concourse.tile as tile
from concourse import library_config

F32 = mybir.dt.float32
F16 = mybir.dt.float16
I16 = mybir.dt.int16
AL = mybir.AluOpType
ACTF = mybir.ActivationFunctionType

NQ = 4
_QCTR = [0]


def pos_packed_dram_ap(t, cols, feat):
    flat = t[:, :].rearrange("a b -> (a b)")
    return flat.rearrange("(c p f) -> p c f", p=128, f=feat)


def merge_windows(windows):
    """Merge (a, so, ww) runs contiguous in both acc cols and pair slots."""
    out = []
    for (a, so, ww) in sorted(windows, key=lambda t: t[1]):
        if out and out[-1][0] + out[-1][2] == a and \
                out[-1][1] + out[-1][2] == so:
            out[-1] = (out[-1][0], out[-1][1], out[-1][2] + ww)
        else:
            out.append((a, so, ww))
    return out


def emit_seg_gathers(nc, pool, table_ap, idx_tile, mask_tile, segs, acc,
                     segw, gtag="g", stream=None, post_seg=None,
                     no_select=False, no_gather=False):
    """Pair-wise gather + select-add into acc [128, C, 32].

    Gathers two segments (each <= segw cols) into one pair tile
    [128, 2, segw, 4, 32] f16, then one fused select pass over the pair.
    Resident mode: idx_tile [128, K] i16 + mask_tile [128, T, 4, 1] f16.
    Streaming mode (stream=(gx_param, m_param, spool)): per-pair DMA.
    """
    stream_col = 0
    i = 0
    nseg = len(segs)
    while i < nseg:
        pair = segs[i:i + 2]
        pw = [s["ncols"] for s in pair]
        assert all(w <= segw for w in pw)
        tot = sum(pw)
        g = pool.tile([128, 2, segw, 4, 32], F16, tag=gtag)
        if stream is not None:
            gx_p, m_p, spool = stream
            idx_ap = spool.tile([128, 2 * segw * 8], I16, tag=gtag + "sx")
            nc.sync.dma_start(
                out=idx_ap[:, 0:tot * 8],
                in_=gx_p[:, stream_col * 8:(stream_col + tot) * 8])
            mk = spool.tile([128, 2 * segw, 4, 1], F16, tag=gtag + "sm")
            nc.sync.dma_start(
                out=mk[:, 0:tot, :, :],
                in_=m_p[:, stream_col:stream_col + tot, :, None])
            idx_base = 0
            mk_base = 0
        else:
            idx_ap = idx_tile
            mk = mask_tile
            idx_base = stream_col * 8
            mk_base = stream_col
        off = 0
        for k, s in enumerate(pair):
            w = pw[k]
            if not no_gather:
                nc.gpsimd.dma_gather(
                    out_ap=g[:, k, 0:w, :, :]
                        .rearrange("p w a b -> p w (a b)"),
                    in_ap=table_ap,
                    idxs_ap=idx_ap[:, idx_base + off * 8:
                                   idx_base + (off + w) * 8],
                    num_idxs=w * 128,
                    num_idxs_reg=w * 128,
                    elem_size=128,
                    single_packet=False,
                    queue_num=_QCTR[0] % NQ,
                )
                _QCTR[0] += 1
            off += w
        gp2 = g[:, :, :, :, :].rearrange("p k w a b -> p (k w) a b")
        if no_select:
            nc.vector.tensor_tensor(
                out=acc[:, 0:1, :], in0=acc[:, 0:1, :],
                in1=gp2[:, 0, 0, :][:, None, :], op=AL.add)
        else:
            off = 0
            for k, s in enumerate(pair):
                w = pw[k]
                mslice = mk[:, mk_base + off:mk_base + off + w, :, :]
                nc.vector.tensor_tensor(
                    out=g[:, k, 0:w, :, :], in0=g[:, k, 0:w, :, :],
                    in1=mslice.to_broadcast([128, w, 4, 32]),
                    op=AL.mult)
                off += w
            nc.vector.tensor_tensor(
                out=gp2[:, :, 0:2, :], in0=gp2[:, :, 0:2, :],
                in1=gp2[:, :, 2:4, :], op=AL.add)
            nc.vector.tensor_tensor(
                out=gp2[:, :, 0, :], in0=gp2[:, :, 0, :],
                in1=gp2[:, :, 1, :], op=AL.add)
            wins = []
            for k, s in enumerate(pair):
                for (r, a, ww, so) in s["windows"]:
                    wins.append((a, k * segw + so, ww))
            for (a, so, ww) in merge_windows(wins):
                nc.vector.tensor_tensor(
                    out=acc[:, a:a + ww, :], in0=acc[:, a:a + ww, :],
                    in1=gp2[:, so:so + ww, 0, :], op=AL.add)
        if post_seg is not None:
            for k in range(len(pair)):
                post_seg(i + k)
        stream_col += tot
        i += 2


def emit_tl_matmul(nc, pool, psum_pool, src, cols, fin, wtiles, bias_tile,
                   out, act_func, scale=1.0, ztag="zt", ytag="yt",
                   src_dram=None, dt=F16, dt_out=None):
    """out[:, :, 0:32] = act(scale * (src @ W) + bias) via TL transform."""
    if dt_out is None:
        dt_out = dt
    cc = 512 // fin
    nk = fin // 32
    nch = (cols + cc - 1) // cc
    for j in range(nch):
        c0 = j * cc
        w = min(cc, cols - c0)
        if src_dram is not None:
            st = pool.tile([128, cc, fin], F32, tag=ztag + "ld")
            nc.sync.dma_start(out=st[:, 0:w, :], in_=src_dram(c0, w))
            src_ap = st[:, 0:w, :]
        else:
            src_ap = src[:, c0:c0 + w, :]
        zt = pool.tile([128, 512], dt, tag=ztag)
        nc.vector.transpose(out=zt[:, 0:w * fin],
                            in_=src_ap.rearrange("p c f -> p (c f)"))
        ps = psum_pool.tile([128, cc * 32], F32, space="PSUM", tag="ps")
        for k in range(nk):
            rhs = zt[:, 0:w * fin].rearrange("p (c k f) -> p c k f", k=nk,
                                             f=32)[:, :, k, :]
            nc.tensor.matmul(out=ps[:, 0:w * 32], lhsT=wtiles[k][:, :],
                             rhs=rhs, start=(k == 0), stop=(k == nk - 1))
        yt = pool.tile([128, cc * 32], dt_out, tag=ytag)
        nc.scalar.activation(out=yt[:, 0:w * 32], in_=ps[:, 0:w * 32],
                             func=act_func, bias=bias_tile, scale=scale)
        nc.vector.transpose(out=out[:, c0:c0 + w, :]
                            .rearrange("p c f -> p (c f)"),
                            in_=yt[:, 0:w * 32])


def build_program(cfg, ablate=()):
    ab = set(ablate)
    C8 = cfg["C8"]
    NC = 8
    POS8 = C8 * 128
    GRP8 = POS8 // 4
    TA, TP, TN = cfg["TA"], cfg["TP"], cfg["TN"]
    TH = TC = C8
    segA, segP, segN = cfg["segA"], cfg["segP"], cfg["segN"]
    segH, segC = cfg["segH"], cfg["segC"]
    NSTEP = cfg["NSTEP"]
    SEGW = max(s["ncols"] for s in segP + segN + segA + segH + segC)

    nc = bacc.Bacc("TRN2", target_bir_lowering=False, debug=False,
                   num_devices=NC, num_swdge_queues=NQ)
    _QCTR[0] = 0

    def param(name, shape, dt=F32, out=False):
        return nc.declare_dram_parameter(name, list(shape), dt, isOutput=out)

    ht = param("ht", [POS8, 128])
    degA = param("degA", [128, C8, 1])
    degP = param("degP", [128, C8, 1])
    degN = param("degN", [128, C8, 1])
    wib = param("wib", [4, 128, 128])
    wpos = param("wpos", [128, 128], F16)
    wneg = param("wneg", [128, 128], F16)
    cwb = param("cwb", [2, 128, 128], F16)
    bi32 = param("bi32", [128, 32])
    bstkp = param("bstkp", [128, 1])
    bstkn = param("bstkn", [128, 1])
    cbstk = param("cbstk", [128, 1])
    wtp = param("wtp", [128, 1])
    wtn = param("wtn", [128, 1])
    tgrid = param("tgrid", [128, 4 * NSTEP])
    lng = param("lng", [128, 32])
    lnb = param("lnb", [128, 32])
    gxA = param("gxA", [128, TA * 8], I16)
    mA = param("mA", [128, TA, 4], F16)
    gxP = param("gxP", [128, TP * 8], I16)
    mP = param("mP", [128, TP, 4], F16)
    gxN = param("gxN", [128, TN * 8], I16)
    mN = param("mN", [128, TN, 4], F16)
    gxHp = param("gxHp", [128, TH * 8], I16)
    mHp = param("mHp", [128, TH, 4], F16)
    gxHn = param("gxHn", [128, TH * 8], I16)
    mHn = param("mHn", [128, TH, 4], F16)
    gxCp = param("gxCp", [128, TC * 8], I16)
    mCp = param("mCp", [128, TC, 4], F16)
    gxCn = param("gxCn", [128, TC * 8], I16)
    mCn = param("mCn", [128, TC, 4], F16)
    out_o = param("o", [POS8, 32], out=True)

    aginA = nc.dram_tensor("aginA", [GRP8, 128], F16)
    tableA = nc.dram_tensor("tableA", [8 * GRP8, 128], F16,
                            addr_space="Shared")
    aginH = nc.dram_tensor("aginH", [GRP8, 128], F16)
    tableH = nc.dram_tensor("tableH", [8 * GRP8, 128], F16,
                            addr_space="Shared")
    aginP = nc.dram_tensor("aginP", [GRP8, 128], F16)
    tableP = nc.dram_tensor("tableP", [8 * GRP8, 128], F16,
                            addr_space="Shared")
    aginN = nc.dram_tensor("aginN", [GRP8, 128], F16)
    tableN = nc.dram_tensor("tableN", [8 * GRP8, 128], F16,
                            addr_space="Shared")
    tableCp = nc.dram_tensor("tableCp", [8 * GRP8, 128], F16,
                             addr_space="Shared")
    tableCn = nc.dram_tensor("tableCn", [8 * GRP8, 128], F16,
                             addr_space="Shared")

    RG_ALL = [list(range(NC))]

    with tile.TileContext(nc) as tc:
        with (
            tc.tile_pool(name="const", bufs=1) as cp,
            tc.tile_pool(name="state", bufs=1) as sp,
            tc.tile_pool(name="work", bufs=2) as wp,
            tc.tile_pool(name="gpool", bufs=7) as gp,
            tc.tile_pool(name="psum", bufs=2, space="PSUM") as pp,
        ):
            nc.gpsimd.load_library(library_config.mlp)

            # ---- constants ----
            wib_t = []
            for k in range(4):
                t = cp.tile([128, 128], F32, tag=f"wib{k}")
                nc.sync.dma_start(out=t[:], in_=wib[k, :, :])
                wib_t.append(t)
            wpos_t = cp.tile([128, 128], F16, tag="wpos")
            nc.sync.dma_start(out=wpos_t[:], in_=wpos[:, :])
            wneg_t = cp.tile([128, 128], F16, tag="wneg")
            nc.sync.dma_start(out=wneg_t[:], in_=wneg[:, :])
            cwb_t = []
            for k in range(2):
                t = cp.tile([128, 128], F16, tag=f"cwb{k}")
                nc.sync.dma_start(out=t[:], in_=cwb[k, :, :])
                cwb_t.append(t)

            bi32_t = cp.tile([128, 1, 32], F32, tag="bi32")
            nc.sync.dma_start(out=bi32_t[:], in_=bi32[:, None, :])
            bstkp_t = cp.tile([128, 1], F32, tag="bstkp")
            nc.sync.dma_start(out=bstkp_t[:], in_=bstkp[:, :])
            bstkn_t = cp.tile([128, 1], F32, tag="bstkn")
            nc.sync.dma_start(out=bstkn_t[:], in_=bstkn[:, :])
            cbstk_t = cp.tile([128, 1], F32, tag="cbstk")
            nc.sync.dma_start(out=cbstk_t[:], in_=cbstk[:, :])
            lng_t = cp.tile([128, 1, 32], F32, tag="lng")
            nc.sync.dma_start(out=lng_t[:], in_=lng[:, None, :])
            lnb_t = cp.tile([128, 1, 32], F32, tag="lnb")
            nc.sync.dma_start(out=lnb_t[:], in_=lnb[:, None, :])

            tg_t = cp.tile([128, 4 * NSTEP], F32, tag="tg")
            nc.sync.dma_start(out=tg_t[:], in_=tgrid[:, :])
            wtp_t = cp.tile([128, 1], F32, tag="wtp")
            nc.sync.dma_start(out=wtp_t[:], in_=wtp[:, :])
            wtn_t = cp.tile([128, 1], F32, tag="wtn")
            nc.sync.dma_start(out=wtn_t[:], in_=wtn[:, :])
            gates_p = cp.tile([128, 4 * NSTEP], F32, tag="gp")
            nc.vector.tensor_scalar_mul(gates_p[:], tg_t[:], wtp_t[:])
            nc.scalar.activation(out=gates_p[:], in_=gates_p[:],
                                 func=ACTF.Sigmoid)
            gates_n = cp.tile([128, 4 * NSTEP], F32, tag="gn")
            nc.vector.tensor_scalar_mul(gates_n[:], tg_t[:], wtn_t[:])
            nc.scalar.activation(out=gates_n[:], in_=gates_n[:],
                                 func=ACTF.Sigmoid)

            dinvA2 = cp.tile([128, C8, 1], F32, tag="dA")
            nc.sync.dma_start(out=dinvA2[:], in_=degA[:, :, :])
            dinvP2 = cp.tile([128, C8, 1], F32, tag="dP")
            nc.sync.dma_start(out=dinvP2[:], in_=degP[:, :, :])
            dinvN2 = cp.tile([128, C8, 1], F32, tag="dN")
            nc.sync.dma_start(out=dinvN2[:], in_=degN[:, :, :])

            # ---- state (fp16 select/accumulate datapath) ----
            x_p = sp.tile([128, C8, 32], F32, tag="x_p")
            x_n = sp.tile([128, C8, 32], F32, tag="x_n")
            ksum_p = sp.tile([128, C8, 32], F16, tag="ksum_p")
            ksum_n = sp.tile([128, C8, 32], F16, tag="ksum_n")
            v_p = sp.tile([128, C8, 32], F16, tag="v_p")
            v_n = sp.tile([128, C8, 32], F16, tag="v_n")
            acc = sp.tile([128, C8, 32], F16, tag="acc")

            # ================= PHASE A =================
            with tc.tile_pool(name="phas", bufs=2) as as_pool:
                ht_ap = pos_packed_dram_ap(ht, C8, 128)

                def ht_chunk(c0, w):
                    return ht_ap[:, c0:c0 + w, :]

                xs1 = ksum_p[:, 0:C8, :]
                emit_tl_matmul(nc, as_pool, pp, None, C8, 128, wib_t, 0.0,
                               xs1, ACTF.Identity, src_dram=ht_chunk,
                               dt=F32, dt_out=F16)
                nc.sync.dma_start(out=pos_packed_dram_ap(aginA, C8, 32),
                                  in_=xs1[:, :, :])
                nc.gpsimd.collective_compute(
                    "AllGather", AL.bypass, replica_groups=RG_ALL,
                    ins=[aginA[:, :]], outs=[tableA[:, :]])

                accA = acc[:, 0:C8, :]
                nc.vector.tensor_tensor(
                    out=accA[:, :, :], in0=xs1[:, :, :],
                    in1=dinvA2[:, :, :].to_broadcast([128, C8, 32]),
                    op=AL.mult)
                with tc.tile_pool(name="phast", bufs=4) as astr:
                    emit_seg_gathers(nc, gp, tableA[:, :], None, None,
                                     segA, accA, SEGW,
                                     stream=(gxA, mA, astr))
                nc.vector.tensor_tensor(
                    out=accA[:, :, :], in0=accA[:, :, :],
                    in1=bi32_t[:, :, :].to_broadcast([128, C8, 32]),
                    op=AL.add)
                nc.vector.tensor_scalar(
                    out=v_p[:, :, :], in0=accA[:, :, :], scalar1=0.0,
                    scalar2=None, op0=AL.max)
                nc.sync.dma_start(out=pos_packed_dram_ap(aginH, C8, 32),
                                  in_=v_p[:, :, :])
                nc.gpsimd.collective_compute(
                    "AllGather", AL.bypass, replica_groups=RG_ALL,
                    ins=[aginH[:, :]], outs=[tableH[:, :]])

            # ================= x0 redistribution =================
            with tc.tile_pool(name="phh", bufs=2) as hp:
                nc.vector.memset(x_p[:, :, :], 0.0)
                emit_seg_gathers(nc, gp, tableH[:, :], None, None,
                                 segH, x_p, SEGW, stream=(gxHp, mHp, hp))
                nc.vector.memset(x_n[:, :, :], 0.0)
                emit_seg_gathers(nc, gp, tableH[:, :], None, None,
                                 segH, x_n, SEGW, stream=(gxHn, mHn, hp))

            aginP_ap = pos_packed_dram_ap(aginP, C8, 32)
            aginN_ap = pos_packed_dram_ap(aginN, C8, 32)
            nc.vector.tensor_copy(out=v_p[:, :, :], in_=x_p[:, :, :])
            nc.sync.dma_start(out=aginP_ap, in_=v_p[:, :, :])
            nc.gpsimd.collective_compute(
                "AllGather", AL.bypass, replica_groups=RG_ALL,
                ins=[aginP[:, :]], outs=[tableP[:, :]])
            nc.vector.tensor_copy(out=v_n[:, :, :], in_=x_n[:, :, :])
            nc.sync.dma_start(out=aginN_ap, in_=v_n[:, :, :])
            nc.gpsimd.collective_compute(
                "AllGather", AL.bypass, replica_groups=RG_ALL,
                ins=[aginN[:, :]], outs=[tableN[:, :]])

            # ---- phase-B idx/mask streamed per pair ----
            with tc.tile_pool(name="phbs", bufs=6) as bsp:
                # ================= PHASE B =================
                NCH = (C8 + 15) // 16

                def mk_ready(segs):
                    last_touch = [0] * NCH
                    for si, sg in enumerate(segs):
                        for (r, a, w_, so) in sg["windows"]:
                            for ci in range(a // 16,
                                            min(NCH, (a + w_ + 15) // 16)):
                                last_touch[ci] = max(last_touch[ci], si)
                    ready_after = [[] for _ in range(len(segs))]
                    for ci, si in enumerate(last_touch):
                        ready_after[si].append(ci)
                    return ready_after

                readyP = mk_ready(segP)
                readyN = mk_ready(segN)
                coef = [H_ODE * 0.5, H_ODE * 0.5, H_ODE]

                sysP = dict(table=tableP, tableC=tableCp, agin=aginP,
                            agin_ap=aginP_ap, gates=gates_p, wblk=wpos_t,
                            bstk=bstkp_t, dinv2=dinvP2, x=x_p, ksum=ksum_p,
                            v=v_p, gx=gxP, m=mP, seg=segP,
                            ready=readyP)
                sysN = dict(table=tableN, tableC=tableCn, agin=aginN,
                            agin_ap=aginN_ap, gates=gates_n, wblk=wneg_t,
                            bstk=bstkn_t, dinv2=dinvN2, x=x_n, ksum=ksum_n,
                            v=v_n, gx=gxN, m=mN, seg=segN,
                            ready=readyN)

                for i in range(NSTEP):
                    for s in range(4):
                        last_sub = (i == NSTEP - 1 and s == 3)
                        for sy in (sysP, sysN):
                            x, ksum, v = sy["x"], sy["ksum"], sy["v"]
                            nc.vector.tensor_tensor(
                                out=acc[:, :, :], in0=v[:, :, :],
                                in1=sy["dinv2"][:, :, :]
                                    .to_broadcast([128, C8, 32]),
                                op=AL.mult)
                            gsc = sy["gates"][:, 4 * i + s:4 * i + s + 1]
                            gb = wp.tile([128, 1], F32, tag="gb")
                            nc.vector.tensor_tensor(out=gb[:, :],
                                                    in0=sy["bstk"][:, :],
                                                    in1=gsc, op=AL.mult)

                            def finalize_chunk(ci, sy=sy, x=x, ksum=ksum,
                                               v=v, gsc=gsc, gb=gb, s=s,
                                               last_sub=last_sub):
                                c0 = 16 * ci
                                w_ = min(16, C8 - c0)
                                kc = acc[:, c0:c0 + w_, :]
                                if "nofin" in ab:
                                    nc.sync.dma_start(
                                        out=sy["agin_ap"][:, c0:c0 + w_, :],
                                        in_=v[:, c0:c0 + w_, :])
                                    return
                                emit_tl_matmul(
                                    nc, wp, pp, acc[:, c0:c0 + w_, :], w_,
                                    32, [sy["wblk"]], gb[:, :], kc,
                                    ACTF.Relu, scale=gsc)
                                if s == 0:
                                    nc.vector.tensor_copy(
                                        out=ksum[:, c0:c0 + w_, :], in_=kc)
                                elif s in (1, 2):
                                    nc.vector.scalar_tensor_tensor(
                                        out=ksum[:, c0:c0 + w_, :], in0=kc,
                                        scalar=2.0,
                                        in1=ksum[:, c0:c0 + w_, :],
                                        op0=AL.mult, op1=AL.add)
                                else:
                                    nc.vector.tensor_tensor(
                                        out=ksum[:, c0:c0 + w_, :],
                                        in0=ksum[:, c0:c0 + w_, :],
                                        in1=kc, op=AL.add)
                                if s < 3:
                                    nc.vector.scalar_tensor_tensor(
                                        out=v[:, c0:c0 + w_, :], in0=kc,
                                        scalar=float(coef[s]),
                                        in1=x[:, c0:c0 + w_, :],
                                        op0=AL.mult, op1=AL.add)
                                else:
                                    nc.vector.scalar_tensor_tensor(
                                        out=x[:, c0:c0 + w_, :],
                                        in0=ksum[:, c0:c0 + w_, :],
                                        scalar=H_ODE / 6.0,
                                        in1=x[:, c0:c0 + w_, :],
                                        op0=AL.mult, op1=AL.add)
                                    nc.vector.tensor_copy(
                                        out=v[:, c0:c0 + w_, :],
                                        in_=x[:, c0:c0 + w_, :])
                                nc.sync.dma_start(
                                    out=sy["agin_ap"][:, c0:c0 + w_, :],
                                    in_=v[:, c0:c0 + w_, :])

                            def post_seg(si, sy=sy):
                                for ci in sy["ready"][si]:
                                    finalize_chunk(ci)

                            if "nogather" in ab:
                                for ci in range(NCH):
                                    finalize_chunk(ci)
                            else:
                                emit_seg_gathers(
                                    nc, gp, sy["table"][:, :], None,
                                    None, sy["seg"], acc, SEGW,
                                    stream=(sy["gx"], sy["m"], bsp),
                                    post_seg=post_seg,
                                    no_select=("nosel" in ab),
                                    no_gather=("selgarb" in ab))
                            if "noag" not in ab or last_sub:
                                nc.gpsimd.collective_compute(
                                    "AllGather", AL.bypass,
                                    replica_groups=RG_ALL,
                                    ins=[sy["agin"][:, :]],
                                    outs=[(sy["tableC"] if last_sub
                                           else sy["table"])[:, :]])

                # ================= PHASE C =================
                with tc.tile_pool(name="phc", bufs=2) as cpp:
                    zp = acc
                    zn = ksum_p
                    nc.vector.memset(zp[:, :, :], 0.0)
                    nc.vector.memset(zn[:, :, :], 0.0)
                    with tc.tile_pool(name="phcs", bufs=2) as csp:
                        emit_seg_gathers(nc, gp, tableCp[:, :], None, None,
                                         segC, zp, SEGW,
                                         stream=(gxCp, mCp, csp))
                        emit_seg_gathers(nc, gp, tableCn[:, :], None, None,
                                         segC, zn, SEGW,
                                         stream=(gxCn, mCn, csp))
                    zc = x_p
                    cc = 16
                    for j in range((TC + cc - 1) // cc):
                        c0 = j * cc
                        w = min(cc, TC - c0)
                        ztp = cpp.tile([128, 512], F16, tag="ztp")
                        nc.vector.transpose(
                            out=ztp[:, 0:w * 32],
                            in_=zp[:, c0:c0 + w, :]
                                .rearrange("p c f -> p (c f)"))
                        ztn = cpp.tile([128, 512], F16, tag="ztn")
                        nc.vector.transpose(
                            out=ztn[:, 0:w * 32],
                            in_=zn[:, c0:c0 + w, :]
                                .rearrange("p c f -> p (c f)"))
                        ps = pp.tile([128, cc * 32], F32, space="PSUM",
                                     tag="ps")
                        nc.tensor.matmul(
                            out=ps[:, 0:w * 32], lhsT=cwb_t[0][:, :],
                            rhs=ztp[:, 0:w * 32].rearrange(
                                "p (c f) -> p c f", f=32),
                            start=True, stop=False)
                        nc.tensor.matmul(
                            out=ps[:, 0:w * 32], lhsT=cwb_t[1][:, :],
                            rhs=ztn[:, 0:w * 32].rearrange(
                                "p (c f) -> p c f", f=32),
                            start=False, stop=True)
                        yt = cpp.tile([128, cc * 32], F32, tag="yt")
                        nc.scalar.activation(out=yt[:, 0:w * 32],
                                             in_=ps[:, 0:w * 32],
                                             func=ACTF.Identity,
                                             bias=cbstk_t[:, :], scale=1.0)
                        nc.vector.transpose(
                            out=zc[:, c0:c0 + w, :]
                                .rearrange("p c f -> p (c f)"),
                            in_=yt[:, 0:w * 32])
                    # layernorm over the 32 features
                    s1 = cpp.tile([128, TC, 1], F32, tag="s1")
                    nc.vector.tensor_reduce(out=s1[:, :, :],
                                            in_=zc[:, :, :],
                                            axis=mybir.AxisListType.X,
                                            op=AL.add)
                    nc.vector.tensor_scalar_mul(s1[:, :, :], s1[:, :, :],
                                                -1.0 / 32.0)
                    nc.vector.tensor_tensor(
                        out=zc[:, :, :], in0=zc[:, :, :],
                        in1=s1[:, :, :].to_broadcast([128, TC, 32]),
                        op=AL.add)
                    sq = x_n
                    nc.scalar.activation(out=sq[:, :, :], in_=zc[:, :, :],
                                         func=ACTF.Square)
                    v1 = cpp.tile([128, TC, 1], F32, tag="v1")
                    nc.vector.tensor_reduce(out=v1[:, :, :],
                                            in_=sq[:, :, :],
                                            axis=mybir.AxisListType.X,
                                            op=AL.add)
                    nc.vector.tensor_scalar(
                        out=v1[:, :, :], in0=v1[:, :, :],
                        scalar1=1.0 / 32.0,
                        scalar2=1e-5, op0=AL.mult, op1=AL.add)
                    nc.scalar.activation(out=v1[:, :, :], in_=v1[:, :, :],
                                         func=ACTF.Sqrt)
                    nc.vector.reciprocal(v1[:, :, :], v1[:, :, :])
                    nc.vector.tensor_tensor(
                        out=zc[:, :, :], in0=zc[:, :, :],
                        in1=v1[:, :, :].to_broadcast([128, TC, 32]),
                        op=AL.mult)
                    nc.vector.tensor_tensor(
                        out=zc[:, :, :], in0=zc[:, :, :],
                        in1=lng_t[:, :, :].to_broadcast([128, TC, 32]),
                        op=AL.mult)
                    nc.vector.tensor_tensor(
                        out=zc[:, :, :], in0=zc[:, :, :],
                        in1=lnb_t[:, :, :].to_broadcast([128, TC, 32]),
                        op=AL.add)
                    nc.sync.dma_start(out=pos_packed_dram_ap(out_o, TC, 32),
                                      in_=zc[:, :, :])
    nc.compile()
    return nc


# ============================ entry point ============================
_CACHE = {}


def kernel(**inputs):
    from concourse.bass_utils import run_bass_kernel_spmd

    cfg, in_maps, meta = build_all(inputs)
    key = "prog"
    if key not in _CACHE:
        _CACHE[key] = build_program(cfg)
    nc = _CACHE[key]
    br = run_bass_kernel_spmd(nc, in_maps, list(range(8)))
    return assemble_output(br.results, meta["perA"],
                           inputs["H_t"].shape[0]).astype(np.float32)


# revision 13
# speedup vs baseline: 1.1371x; 1.1371x over previous
"""Trainium2 Bass kernel for nn_DynamiSE (GNN message passing + RK4 ODE).

V3 layout: ALL 8 cores run BOTH ODEs (each core owns N/8=12500 nodes of
pos-state and neg-state). Per RK4 substep, each ODE does an 8-rank
AllGather with a Shared-space output table (fast path) and the pos
AllGather overlaps the neg compute. Phase-B gather indices/masks are
SBUF-resident (loaded once after phase A). Gather select runs pair-wise
(two 49-col segments per select pass) with merged window adds; the
select/accumulate datapath is fp16.

Entry point: kernel(**inputs) -> np.ndarray [100000, 32] float32.
"""

import numpy as np

H_ODE = 0.1


# ===================== host-side graph preprocessing =====================

def shard_meta(N, nshards, npos):
    per = N // nshards
    owner = np.minimum(np.arange(N) // per, nshards - 1)
    local = np.arange(N) - owner * per
    return owner, local, per


def degree_sort_positions(dst, owner_of, local_of, nshards, per, npos):
    N = len(owner_of)
    deg_all = np.bincount(dst, minlength=N)
    pos_of = np.empty(N, np.int64)
    for c in range(nshards):
        nodes = np.where(owner_of == c)[0]
        order = nodes[np.argsort(-deg_all[nodes], kind="stable")]
        pos_of[order] = np.arange(len(nodes))
    return pos_of, deg_all


def make_segments(cols_r_glob, seg_cols):
    """Split per-round column counts into gather segments."""
    segs = []
    pend_windows, pend_cols = [], 0

    def flush():
        nonlocal pend_windows, pend_cols
        if pend_windows:
            segs.append({"windows": pend_windows, "ncols": pend_cols})
            pend_windows, pend_cols = [], 0

    for r, cols in enumerate(cols_r_glob):
        if cols >= seg_cols // 2:
            flush()
            a = 0
            while a < cols:
                w = min(seg_cols, cols - a)
                segs.append({"windows": [(r, a, w, 0)], "ncols": w})
                a += w
        else:
            if pend_cols + cols > seg_cols:
                flush()
            pend_windows.append((r, 0, cols, pend_cols))
            pend_cols += cols
    flush()
    return segs


def wrap_indices(rows_stream):
    n = len(rows_stream)
    assert n % 128 == 0
    w = np.zeros((16, n // 16), np.int16)
    idxs = np.arange(n)
    w[idxs % 16, idxs // 16] = rows_stream.astype(np.int16)
    return np.tile(w, (8, 1))


def build_streams(src, dst, owner_of, pos_of, nshards, npos, row_of, sub_of,
                  seg_cols, coef=None):
    """Round/segment gather streams for one edge list over dst shards."""
    grp = npos // 4
    if coef is None:
        coef = np.ones(len(src), np.float32)
    shard_rounds = []
    for c in range(nshards):
        m = owner_of[dst] == c
        s_c = src[m]
        cf_c = coef[m]
        p_c = pos_of[dst[m]]
        order = np.lexsort((np.arange(len(s_c)), p_c))
        ps, ss, cs = p_c[order], s_c[order], cf_c[order]
        first = np.searchsorted(ps, ps)
        rank = np.arange(len(ps)) - first
        maxdeg = int(rank.max()) + 1 if len(rank) else 0
        rounds = []
        for r in range(maxdeg):
            sel = rank == r
            psr, ssr, csr = ps[sel], ss[sel], cs[sel]
            assert np.array_equal(psr, np.arange(len(psr)))
            rounds.append((len(psr), ssr, csr))
        shard_rounds.append(rounds)

    nr = max(len(r) for r in shard_rounds)
    cols_r = [max((len(sr[r][1]) if r < len(sr) else 0)
                  for sr in shard_rounds) for r in range(nr)]
    cols_r = [(n + 127) // 128 for n in cols_r]
    segs = make_segments(cols_r, seg_cols)
    totcols = sum(s["ncols"] for s in segs)

    per_shard = []
    for c in range(nshards):
        zero_row = c * grp + (grp - 1)
        rows_stream = np.full(totcols * 128, zero_row, np.int64)
        subs_stream = np.zeros(totcols * 128, np.int64)
        vals_stream = np.zeros(totcols * 128, np.float32)
        off = 0
        for s in segs:
            for (r, a, w, so) in s["windows"]:
                if r < len(shard_rounds[c]):
                    _, ssr, csr = shard_rounds[c][r]
                    lo, hi = a * 128, min((a + w) * 128, len(ssr))
                    if hi > lo:
                        dstslice = slice(off + so * 128,
                                         off + so * 128 + (hi - lo))
                        rows_stream[dstslice] = row_of[ssr[lo:hi]]
                        subs_stream[dstslice] = sub_of[ssr[lo:hi]]
                        vals_stream[dstslice] = csr[lo:hi]
            off += s["ncols"] * 128
        gidx = wrap_indices(rows_stream)
        masks = np.zeros((128, totcols, 4), np.float16)
        ii = np.arange(totcols * 128)
        masks[ii % 128, ii // 128, subs_stream] = vals_stream
        per_shard.append((gidx, masks))
    return segs, totcols, per_shard


def blockdiag4(W):
    fin, fout = W.shape
    assert fout == 32 and fin % 32 == 0
    tiles = []
    for k in range(fin // 32):
        t = np.zeros((128, 128), np.float32)
        for bi in range(4):
            t[32 * bi:32 * bi + 32, 32 * bi:32 * bi + 32] = \
                W[32 * k:32 * k + 32, :]
        tiles.append(t)
    return np.stack(tiles)


def build_all(inputs, seg_cols=33, nstep=10, seg_cols_b=33):
    N = inputs["H_t"].shape[0]
    NC = 8
    per8 = N // NC
    C8 = (per8 + 4 + 127) // 128
    POS8 = C8 * 128
    GRP8 = POS8 // 4

    edges_all = np.concatenate([inputs["A_pos_t"], inputs["A_neg_t"]],
                               axis=1).astype(np.int64)
    dp = inputs["dA_pos"].astype(np.int64)
    dn = inputs["dA_neg"].astype(np.int64)

    own8, loc8, _ = shard_meta(N, NC, POS8)

    posA, degA_all = degree_sort_positions(edges_all[1], own8, loc8, NC,
                                           per8, POS8)
    rowA = own8 * GRP8 + posA // 4
    subA = posA % 4
    posP, degP_all = degree_sort_positions(dp[1], own8, loc8, NC, per8, POS8)
    rowP = own8 * GRP8 + posP // 4
    subP = posP % 4
    posN, degN_all = degree_sort_positions(dn[1], own8, loc8, NC, per8, POS8)
    rowN = own8 * GRP8 + posN // 4
    subN = posN % 4

    dinvA_all = 1.0 / np.sqrt(1.0 + degA_all)
    dinvP_all = 1.0 / np.sqrt(1.0 + degP_all)
    dinvN_all = 1.0 / np.sqrt(1.0 + degN_all)
    coefA = (dinvA_all[edges_all[0]] * dinvA_all[edges_all[1]]).astype(
        np.float32)
    coefP = (dinvP_all[dp[0]] * dinvP_all[dp[1]]).astype(np.float32)
    coefN = (dinvN_all[dn[0]] * dinvN_all[dn[1]]).astype(np.float32)

    segA, TA, shA = build_streams(edges_all[0], edges_all[1], own8, posA,
                                  NC, POS8, rowA, subA, seg_cols, coef=coefA)
    segP, TP, shP = build_streams(dp[0], dp[1], own8, posP, NC, POS8, rowP,
                                  subP, seg_cols_b, coef=coefP)
    segN, TN, shN = build_streams(dn[0], dn[1], own8, posN, NC, POS8, rowN,
                                  subN, seg_cols_b, coef=coefN)

    node_at_posP = np.full((NC, POS8), -1, np.int64)
    node_at_posP[own8, posP] = np.arange(N)
    node_at_posN = np.full((NC, POS8), -1, np.int64)
    node_at_posN[own8, posN] = np.arange(N)

    segH = make_segments([C8], seg_cols)
    segC = make_segments([C8], seg_cols)

    wib = blockdiag4(np.asarray(inputs["W_init"], np.float32))
    wblk_p = blockdiag4(np.asarray(inputs["W_pos"], np.float32))[0] \
        .astype(np.float16)
    wblk_n = blockdiag4(np.asarray(inputs["W_neg"], np.float32))[0] \
        .astype(np.float16)
    cwb = blockdiag4(np.asarray(inputs["W_comb"], np.float32)) \
        .astype(np.float16)
    bi32 = np.tile(np.asarray(inputs["b_init"], np.float32), (128, 1))
    lng = np.tile(np.asarray(inputs["ln_g"], np.float32), (128, 1))
    lnb = np.tile(np.asarray(inputs["ln_b"], np.float32), (128, 1))
    cbstk = np.tile(np.asarray(inputs["b_comb"], np.float32), 4)[:, None]
    bstk_p = np.tile(np.asarray(inputs["b_pos"], np.float32), 4)[:, None]
    bstk_n = np.tile(np.asarray(inputs["b_neg"], np.float32), 4)[:, None]
    wt_p = np.tile(np.asarray(inputs["wt_pos"], np.float32), 4)[:, None]
    wt_n = np.tile(np.asarray(inputs["wt_neg"], np.float32), 4)[:, None]
    offs = np.array([0.0, 0.5, 0.5, 1.0]) * H_ODE
    tg = (np.arange(nstep)[:, None] * H_ODE + offs[None, :]).reshape(-1)
    tgrid = np.tile(tg.astype(np.float32), (128, 1))

    Ht = np.asarray(inputs["H_t"], np.float32)

    def pos_pack(vals, pos, c, fill=0.0):
        arr = np.full(POS8, fill, np.float32)
        nodes = np.where(own8 == c)[0]
        arr[pos[nodes]] = vals[nodes]
        return arr.reshape(C8, 128, 1).transpose(1, 0, 2).copy()

    in_maps = []
    for c in range(NC):
        nodesA = np.where(own8 == c)[0]
        ordA = nodesA[np.argsort(posA[nodesA])]
        ht_c = np.zeros((POS8, 128), np.float32)
        ht_c[:len(ordA)] = Ht[ordA]
        assert np.array_equal(posA[ordA], np.arange(len(ordA)))

        degA_c = pos_pack(dinvA_all * dinvA_all, posA, c)
        degP_c = pos_pack(dinvP_all * dinvP_all, posP, c)
        degN_c = pos_pack(dinvN_all * dinvN_all, posN, c)

        def h_redist(nap):
            rows = np.zeros(POS8, np.int64)
            subs = np.zeros(POS8, np.int64)
            valid = nap >= 0
            rows[valid] = rowA[nap[valid]]
            subs[valid] = subA[nap[valid]]
            gidx = wrap_indices(rows)
            masks = np.zeros((128, C8, 4), np.float16)
            ii = np.arange(POS8)
            m = np.zeros(POS8, np.float16)
            m[valid] = 1.0
            masks[ii % 128, ii // 128, subs] = m
            return gidx, masks

        gxHp_c, mHp_c = h_redist(node_at_posP[c])
        gxHn_c, mHn_c = h_redist(node_at_posN[c])

        tgt = np.arange(c * per8, (c + 1) * per8)

        def c_gather(row_of, sub_of):
            rows = np.zeros(POS8, np.int64)
            subs = np.zeros(POS8, np.int64)
            rows[:per8] = row_of[tgt]
            subs[:per8] = sub_of[tgt]
            gidx = wrap_indices(rows)
            masks = np.zeros((128, C8, 4), np.float16)
            ii = np.arange(POS8)
            m = np.zeros(POS8, np.float16)
            m[:per8] = 1.0
            masks[ii % 128, ii // 128, subs] = m
            return gidx, masks

        gxCp_c, mCp_c = c_gather(rowP, subP)
        gxCn_c, mCn_c = c_gather(rowN, subN)

        in_maps.append({
            "ht": ht_c, "degA": degA_c, "degP": degP_c, "degN": degN_c,
            "wib": wib, "wpos": wblk_p, "wneg": wblk_n, "cwb": cwb,
            "bi32": bi32, "bstkp": bstk_p, "bstkn": bstk_n, "cbstk": cbstk,
            "wtp": wt_p, "wtn": wt_n, "tgrid": tgrid, "lng": lng, "lnb": lnb,
            "gxA": shA[c][0], "mA": shA[c][1],
            "gxP": shP[c][0], "mP": shP[c][1],
            "gxN": shN[c][0], "mN": shN[c][1],
            "gxHp": gxHp_c, "mHp": mHp_c,
            "gxHn": gxHn_c, "mHn": mHn_c,
            "gxCp": gxCp_c, "mCp": mCp_c,
            "gxCn": gxCn_c, "mCn": mCn_c,
        })

    cfg = {"C8": C8, "TA": TA, "TP": TP, "TN": TN,
           "segA": segA, "segP": segP, "segN": segN,
           "segH": segH, "segC": segC, "NSTEP": nstep}
    meta = {"perA": per8, "cfg": cfg}
    return cfg, in_maps, meta


def assemble_output(results, perA, N):
    outs = [results[c]["o"][:perA] for c in range(8)]
    return np.concatenate(outs, axis=0)[:N]


# ============================ device program ============================

from concourse import bass, bacc, mybir
import concourse.tile as tile
from concourse import library_config

F32 = mybir.dt.float32
F16 = mybir.dt.float16
I16 = mybir.dt.int16
AL = mybir.AluOpType
ACTF = mybir.ActivationFunctionType

NQ = 4
_QCTR = [0]


def pos_packed_dram_ap(t, cols, feat):
    flat = t[:, :].rearrange("a b -> (a b)")
    return flat.rearrange("(c p f) -> p c f", p=128, f=feat)


def merge_windows(windows):
    """Merge (a, so, ww) runs contiguous in both acc cols and pair slots."""
    out = []
    for (a, so, ww) in sorted(windows, key=lambda t: t[1]):
        if out and out[-1][0] + out[-1][2] == a and \
                out[-1][1] + out[-1][2] == so:
            out[-1] = (out[-1][0], out[-1][1], out[-1][2] + ww)
        else:
            out.append((a, so, ww))
    return out


def emit_seg_gathers(nc, pool, table_ap, idx_tile, mask_tile, segs, acc,
                     segw, gtag="g", stream=None, post_seg=None,
                     no_select=False, no_gather=False, qs=4):
    """Quad-wise gather + select-add into acc [128, C, 32].

    Gathers up to `qs` segments (each <= segw cols) into one tile
    [128, qs, segw, 4, 32] f16; per-segment fused mask-mult; tree adds run
    full-width across the quad (gap columns hold garbage, never read);
    window adds merged across contiguous runs.
    Resident mode: idx_tile + mask_tile [128, T, 4, 1] f16.
    Streaming mode (stream=(gx_param, m_param, spool)): per-quad DMA.
    """
    stream_col = 0
    i = 0
    nseg = len(segs)
    while i < nseg:
        quad = segs[i:i + qs]
        pw = [s["ncols"] for s in quad]
        assert all(w <= segw for w in pw)
        tot = sum(pw)
        g = pool.tile([128, qs, segw, 4, 32], F16, tag=gtag)
        if stream is not None:
            gx_p, m_p, spool = stream
            idx_ap = spool.tile([128, qs * segw * 8], I16, tag=gtag + "sx")
            nc.sync.dma_start(
                out=idx_ap[:, 0:tot * 8],
                in_=gx_p[:, stream_col * 8:(stream_col + tot) * 8])
            mk = spool.tile([128, qs * segw, 4, 1], F16, tag=gtag + "sm")
            nc.sync.dma_start(
                out=mk[:, 0:tot, :, :],
                in_=m_p[:, stream_col:stream_col + tot, :, None])
            idx_base = 0
            mk_base = 0
        else:
            idx_ap = idx_tile
            mk = mask_tile
            idx_base = stream_col * 8
            mk_base = stream_col
        off = 0
        for k, s in enumerate(quad):
            w = pw[k]
            if not no_gather:
                nc.gpsimd.dma_gather(
                    out_ap=g[:, k, 0:w, :, :]
                        .rearrange("p w a b -> p w (a b)"),
                    in_ap=table_ap,
                    idxs_ap=idx_ap[:, idx_base + off * 8:
                                   idx_base + (off + w) * 8],
                    num_idxs=w * 128,
                    num_idxs_reg=w * 128,
                    elem_size=128,
                    single_packet=False,
                    queue_num=_QCTR[0] % NQ,
                )
                _QCTR[0] += 1
            off += w
        gp2 = g[:, :, :, :, :].rearrange("p k w a b -> p (k w) a b")
        if no_select:
            nc.vector.tensor_tensor(
                out=acc[:, 0:1, :], in0=acc[:, 0:1, :],
                in1=gp2[:, 0, 0, :][:, None, :], op=AL.add)
        else:
            # per-seg fused mask-mult (runs of full segs share one op)
            off = 0
            k = 0
            while k < len(quad):
                k2 = k
                w = 0
                while k2 < len(quad) and pw[k2] == segw:
                    w += segw
                    k2 += 1
                if k2 == k:
                    w = pw[k]
                    k2 = k + 1
                mslice = mk[:, mk_base + off:mk_base + off + w, :, :]
                nc.vector.tensor_tensor(
                    out=gp2[:, k * segw:k * segw + w, :, :],
                    in0=gp2[:, k * segw:k * segw + w, :, :],
                    in1=mslice.to_broadcast([128, w, 4, 32]),
                    op=AL.mult)
                off += w
                k = k2
            nc.vector.tensor_tensor(
                out=gp2[:, :, 0:2, :], in0=gp2[:, :, 0:2, :],
                in1=gp2[:, :, 2:4, :], op=AL.add)
            nc.vector.tensor_tensor(
                out=gp2[:, :, 0, :], in0=gp2[:, :, 0, :],
                in1=gp2[:, :, 1, :], op=AL.add)
            wins = []
            for k, s in enumerate(quad):
                for (r, a, ww, so) in s["windows"]:
                    wins.append((a, k * segw + so, ww))
            for (a, so, ww) in merge_windows(wins):
                nc.vector.tensor_tensor(
                    out=acc[:, a:a + ww, :], in0=acc[:, a:a + ww, :],
                    in1=gp2[:, so:so + ww, 0, :], op=AL.add)
        if post_seg is not None:
            for k in range(len(quad)):
                post_seg(i + k)
        stream_col += tot
        i += qs


def emit_tl_matmul(nc, pool, psum_pool, src, cols, fin, wtiles, bias_tile,
                   out, act_func, scale=1.0, ztag="zt", ytag="yt",
                   src_dram=None, dt=F16, dt_out=None):
    """out[:, :, 0:32] = act(scale * (src @ W) + bias) via TL transform."""
    if dt_out is None:
        dt_out = dt
    cc = 512 // fin
    nk = fin // 32
    nch = (cols + cc - 1) // cc
    for j in range(nch):
        c0 = j * cc
        w = min(cc, cols - c0)
        if src_dram is not None:
            st = pool.tile([128, cc, fin], F32, tag=ztag + "ld")
            nc.sync.dma_start(out=st[:, 0:w, :], in_=src_dram(c0, w))
            src_ap = st[:, 0:w, :]
        else:
            src_ap = src[:, c0:c0 + w, :]
        zt = pool.tile([128, 512], dt, tag=ztag)
        nc.vector.transpose(out=zt[:, 0:w * fin],
                            in_=src_ap.rearrange("p c f -> p (c f)"))
        ps = psum_pool.tile([128, cc * 32], F32, space="PSUM", tag="ps")
        for k in range(nk):
            rhs = zt[:, 0:w * fin].rearrange("p (c k f) -> p c k f", k=nk,
                                             f=32)[:, :, k, :]
            nc.tensor.matmul(out=ps[:, 0:w * 32], lhsT=wtiles[k][:, :],
                             rhs=rhs, start=(k == 0), stop=(k == nk - 1))
        yt = pool.tile([128, cc * 32], dt_out, tag=ytag)
        nc.scalar.activation(out=yt[:, 0:w * 32], in_=ps[:, 0:w * 32],
                             func=act_func, bias=bias_tile, scale=scale)
        nc.vector.transpose(out=out[:, c0:c0 + w, :]
                            .rearrange("p c f -> p (c f)"),
                            in_=yt[:, 0:w * 32])


def build_program(cfg, ablate=()):
    ab = set(ablate)
    C8 = cfg["C8"]
    NC = 8
    POS8 = C8 * 128
    GRP8 = POS8 // 4
    TA, TP, TN = cfg["TA"], cfg["TP"], cfg["TN"]
    TH = TC = C8
    segA, segP, segN = cfg["segA"], cfg["segP"], cfg["segN"]
    segH, segC = cfg["segH"], cfg["segC"]
    NSTEP = cfg["NSTEP"]
    SEGW = max(s["ncols"] for s in segP + segN + segA + segH + segC)

    nc = bacc.Bacc("TRN2", target_bir_lowering=False, debug=False,
                   num_devices=NC, num_swdge_queues=NQ)
    _QCTR[0] = 0

    def param(name, shape, dt=F32, out=False):
        return nc.declare_dram_parameter(name, list(shape), dt, isOutput=out)

    ht = param("ht", [POS8, 128])
    degA = param("degA", [128, C8, 1])
    degP = param("degP", [128, C8, 1])
    degN = param("degN", [128, C8, 1])
    wib = param("wib", [4, 128, 128])
    wpos = param("wpos", [128, 128], F16)
    wneg = param("wneg", [128, 128], F16)
    cwb = param("cwb", [2, 128, 128], F16)
    bi32 = param("bi32", [128, 32])
    bstkp = param("bstkp", [128, 1])
    bstkn = param("bstkn", [128, 1])
    cbstk = param("cbstk", [128, 1])
    wtp = param("wtp", [128, 1])
    wtn = param("wtn", [128, 1])
    tgrid = param("tgrid", [128, 4 * NSTEP])
    lng = param("lng", [128, 32])
    lnb = param("lnb", [128, 32])
    gxA = param("gxA", [128, TA * 8], I16)
    mA = param("mA", [128, TA, 4], F16)
    gxP = param("gxP", [128, TP * 8], I16)
    mP = param("mP", [128, TP, 4], F16)
    gxN = param("gxN", [128, TN * 8], I16)
    mN = param("mN", [128, TN, 4], F16)
    gxHp = param("gxHp", [128, TH * 8], I16)
    mHp = param("mHp", [128, TH, 4], F16)
    gxHn = param("gxHn", [128, TH * 8], I16)
    mHn = param("mHn", [128, TH, 4], F16)
    gxCp = param("gxCp", [128, TC * 8], I16)
    mCp = param("mCp", [128, TC, 4], F16)
    gxCn = param("gxCn", [128, TC * 8], I16)
    mCn = param("mCn", [128, TC, 4], F16)
    out_o = param("o", [POS8, 32], out=True)

    aginA = nc.dram_tensor("aginA", [GRP8, 128], F16)
    tableA = nc.dram_tensor("tableA", [8 * GRP8, 128], F16,
                            addr_space="Shared")
    aginH = nc.dram_tensor("aginH", [GRP8, 128], F16)
    tableH = nc.dram_tensor("tableH", [8 * GRP8, 128], F16,
                            addr_space="Shared")
    aginP = nc.dram_tensor("aginP", [GRP8, 128], F16)
    tableP = nc.dram_tensor("tableP", [8 * GRP8, 128], F16,
                            addr_space="Shared")
    aginN = nc.dram_tensor("aginN", [GRP8, 128], F16)
    tableN = nc.dram_tensor("tableN", [8 * GRP8, 128], F16,
                            addr_space="Shared")
    tableCp = nc.dram_tensor("tableCp", [8 * GRP8, 128], F16,
                             addr_space="Shared")
    tableCn = nc.dram_tensor("tableCn", [8 * GRP8, 128], F16,
                             addr_space="Shared")

    RG_ALL = [list(range(NC))]

    with tile.TileContext(nc) as tc:
        with (
            tc.tile_pool(name="const", bufs=1) as cp,
            tc.tile_pool(name="state", bufs=1) as sp,
            tc.tile_pool(name="work", bufs=2) as wp,
            tc.tile_pool(name="gpool", bufs=3) as gp,
            tc.tile_pool(name="psum", bufs=2, space="PSUM") as pp,
        ):
            nc.gpsimd.load_library(library_config.mlp)

            # ---- constants ----
            wib_t = []
            for k in range(4):
                t = cp.tile([128, 128], F32, tag=f"wib{k}")
                nc.sync.dma_start(out=t[:], in_=wib[k, :, :])
                wib_t.append(t)
            wpos_t = cp.tile([128, 128], F16, tag="wpos")
            nc.sync.dma_start(out=wpos_t[:], in_=wpos[:, :])
            wneg_t = cp.tile([128, 128], F16, tag="wneg")
            nc.sync.dma_start(out=wneg_t[:], in_=wneg[:, :])
            cwb_t = []
            for k in range(2):
                t = cp.tile([128, 128], F16, tag=f"cwb{k}")
                nc.sync.dma_start(out=t[:], in_=cwb[k, :, :])
                cwb_t.append(t)

            bi32_t = cp.tile([128, 1, 32], F32, tag="bi32")
            nc.sync.dma_start(out=bi32_t[:], in_=bi32[:, None, :])
            bstkp_t = cp.tile([128, 1], F32, tag="bstkp")
            nc.sync.dma_start(out=bstkp_t[:], in_=bstkp[:, :])
            bstkn_t = cp.tile([128, 1], F32, tag="bstkn")
            nc.sync.dma_start(out=bstkn_t[:], in_=bstkn[:, :])
            cbstk_t = cp.tile([128, 1], F32, tag="cbstk")
            nc.sync.dma_start(out=cbstk_t[:], in_=cbstk[:, :])
            lng_t = cp.tile([128, 1, 32], F32, tag="lng")
            nc.sync.dma_start(out=lng_t[:], in_=lng[:, None, :])
            lnb_t = cp.tile([128, 1, 32], F32, tag="lnb")
            nc.sync.dma_start(out=lnb_t[:], in_=lnb[:, None, :])

            tg_t = cp.tile([128, 4 * NSTEP], F32, tag="tg")
            nc.sync.dma_start(out=tg_t[:], in_=tgrid[:, :])
            wtp_t = cp.tile([128, 1], F32, tag="wtp")
            nc.sync.dma_start(out=wtp_t[:], in_=wtp[:, :])
            wtn_t = cp.tile([128, 1], F32, tag="wtn")
            nc.sync.dma_start(out=wtn_t[:], in_=wtn[:, :])
            gates_p = cp.tile([128, 4 * NSTEP], F32, tag="gp")
            nc.vector.tensor_scalar_mul(gates_p[:], tg_t[:], wtp_t[:])
            nc.scalar.activation(out=gates_p[:], in_=gates_p[:],
                                 func=ACTF.Sigmoid)
            gates_n = cp.tile([128, 4 * NSTEP], F32, tag="gn")
            nc.vector.tensor_scalar_mul(gates_n[:], tg_t[:], wtn_t[:])
            nc.scalar.activation(out=gates_n[:], in_=gates_n[:],
                                 func=ACTF.Sigmoid)

            dinvA2 = cp.tile([128, C8, 1], F32, tag="dA")
            nc.sync.dma_start(out=dinvA2[:], in_=degA[:, :, :])
            dinvP2 = cp.tile([128, C8, 1], F32, tag="dP")
            nc.sync.dma_start(out=dinvP2[:], in_=degP[:, :, :])
            dinvN2 = cp.tile([128, C8, 1], F32, tag="dN")
            nc.sync.dma_start(out=dinvN2[:], in_=degN[:, :, :])

            # ---- state (fp16 select/accumulate datapath) ----
            x_p = sp.tile([128, C8, 32], F32, tag="x_p")
            x_n = sp.tile([128, C8, 32], F32, tag="x_n")
            ksum_p = sp.tile([128, C8, 32], F16, tag="ksum_p")
            ksum_n = sp.tile([128, C8, 32], F16, tag="ksum_n")
            v_p = sp.tile([128, C8, 32], F16, tag="v_p")
            v_n = sp.tile([128, C8, 32], F16, tag="v_n")
            acc = sp.tile([128, C8, 32], F16, tag="acc")

            # ================= PHASE A =================
            with tc.tile_pool(name="phas", bufs=2) as as_pool:
                ht_ap = pos_packed_dram_ap(ht, C8, 128)

                def ht_chunk(c0, w):
                    return ht_ap[:, c0:c0 + w, :]

                xs1 = ksum_p[:, 0:C8, :]
                emit_tl_matmul(nc, as_pool, pp, None, C8, 128, wib_t, 0.0,
                               xs1, ACTF.Identity, src_dram=ht_chunk,
                               dt=F32, dt_out=F16)
                nc.sync.dma_start(out=pos_packed_dram_ap(aginA, C8, 32),
                                  in_=xs1[:, :, :])
                nc.gpsimd.collective_compute(
                    "AllGather", AL.bypass, replica_groups=RG_ALL,
                    ins=[aginA[:, :]], outs=[tableA[:, :]])

                accA = acc[:, 0:C8, :]
                nc.vector.tensor_tensor(
                    out=accA[:, :, :], in0=xs1[:, :, :],
                    in1=dinvA2[:, :, :].to_broadcast([128, C8, 32]),
                    op=AL.mult)
                with tc.tile_pool(name="phast", bufs=4) as astr:
                    emit_seg_gathers(nc, gp, tableA[:, :], None, None,
                                     segA, accA, SEGW,
                                     stream=(gxA, mA, astr))
                nc.vector.tensor_tensor(
                    out=accA[:, :, :], in0=accA[:, :, :],
                    in1=bi32_t[:, :, :].to_broadcast([128, C8, 32]),
                    op=AL.add)
                nc.vector.tensor_scalar(
                    out=v_p[:, :, :], in0=accA[:, :, :], scalar1=0.0,
                    scalar2=None, op0=AL.max)
                nc.sync.dma_start(out=pos_packed_dram_ap(aginH, C8, 32),
                                  in_=v_p[:, :, :])
                nc.gpsimd.collective_compute(
                    "AllGather", AL.bypass, replica_groups=RG_ALL,
                    ins=[aginH[:, :]], outs=[tableH[:, :]])

            # ================= x0 redistribution =================
            with tc.tile_pool(name="phh", bufs=2) as hp:
                nc.vector.memset(x_p[:, :, :], 0.0)
                emit_seg_gathers(nc, gp, tableH[:, :], None, None,
                                 segH, x_p, SEGW, stream=(gxHp, mHp, hp))
                nc.vector.memset(x_n[:, :, :], 0.0)
                emit_seg_gathers(nc, gp, tableH[:, :], None, None,
                                 segH, x_n, SEGW, stream=(gxHn, mHn, hp))

            aginP_ap = pos_packed_dram_ap(aginP, C8, 32)
            aginN_ap = pos_packed_dram_ap(aginN, C8, 32)
            nc.vector.tensor_copy(out=v_p[:, :, :], in_=x_p[:, :, :])
            nc.sync.dma_start(out=aginP_ap, in_=v_p[:, :, :])
            nc.gpsimd.collective_compute(
                "AllGather", AL.bypass, replica_groups=RG_ALL,
                ins=[aginP[:, :]], outs=[tableP[:, :]])
            nc.vector.tensor_copy(out=v_n[:, :, :], in_=x_n[:, :, :])
            nc.sync.dma_start(out=aginN_ap, in_=v_n[:, :, :])
            nc.gpsimd.collective_compute(
                "AllGather", AL.bypass, replica_groups=RG_ALL,
                ins=[aginN[:, :]], outs=[tableN[:, :]])

            # ---- phase-B idx/mask streamed per pair ----
            with tc.tile_pool(name="phbs", bufs=3) as bsp:
                # ================= PHASE B =================
                NCH = (C8 + 15) // 16

                def mk_ready(segs):
                    last_touch = [0] * NCH
                    for si, sg in enumerate(segs):
                        for (r, a, w_, so) in sg["windows"]:
                            for ci in range(a // 16,
                                            min(NCH, (a + w_ + 15) // 16)):
                                last_touch[ci] = max(last_touch[ci], si)
                    ready_after = [[] for _ in range(len(segs))]
                    for ci, si in enumerate(last_touch):
                        ready_after[si].append(ci)
                    return ready_after

                readyP = mk_ready(segP)
                readyN = mk_ready(segN)
                coef = [H_ODE * 0.5, H_ODE * 0.5, H_ODE]

                sysP = dict(table=tableP, tableC=tableCp, agin=aginP,
                            agin_ap=aginP_ap, gates=gates_p, wblk=wpos_t,
                            bstk=bstkp_t, dinv2=dinvP2, x=x_p, ksum=ksum_p,
                            v=v_p, gx=gxP, m=mP, seg=segP,
                            ready=readyP)
                sysN = dict(table=tableN, tableC=tableCn, agin=aginN,
                            agin_ap=aginN_ap, gates=gates_n, wblk=wneg_t,
                            bstk=bstkn_t, dinv2=dinvN2, x=x_n, ksum=ksum_n,
                            v=v_n, gx=gxN, m=mN, seg=segN,
                            ready=readyN)

                for i in range(NSTEP):
                    for s in range(4):
                        last_sub = (i == NSTEP - 1 and s == 3)
                        for sy in (sysP, sysN):
                            x, ksum, v = sy["x"], sy["ksum"], sy["v"]
                            nc.vector.tensor_tensor(
                                out=acc[:, :, :], in0=v[:, :, :],
                                in1=sy["dinv2"][:, :, :]
                                    .to_broadcast([128, C8, 32]),
                                op=AL.mult)
                            gsc = sy["gates"][:, 4 * i + s:4 * i + s + 1]
                            gb = wp.tile([128, 1], F32, tag="gb")
                            nc.vector.tensor_tensor(out=gb[:, :],
                                                    in0=sy["bstk"][:, :],
                                                    in1=gsc, op=AL.mult)

                            def finalize_chunk(ci, sy=sy, x=x, ksum=ksum,
                                               v=v, gsc=gsc, gb=gb, s=s,
                                               last_sub=last_sub):
                                c0 = 16 * ci
                                w_ = min(16, C8 - c0)
                                kc = acc[:, c0:c0 + w_, :]
                                if "nofin" in ab:
                                    nc.sync.dma_start(
                                        out=sy["agin_ap"][:, c0:c0 + w_, :],
                                        in_=v[:, c0:c0 + w_, :])
                                    return
                                emit_tl_matmul(
                                    nc, wp, pp, acc[:, c0:c0 + w_, :], w_,
                                    32, [sy["wblk"]], gb[:, :], kc,
                                    ACTF.Relu, scale=gsc)
                                if s == 0:
                                    nc.vector.tensor_copy(
                                        out=ksum[:, c0:c0 + w_, :], in_=kc)
                                elif s in (1, 2):
                                    nc.vector.scalar_tensor_tensor(
                                        out=ksum[:, c0:c0 + w_, :], in0=kc,
                                        scalar=2.0,
                                        in1=ksum[:, c0:c0 + w_, :],
                                        op0=AL.mult, op1=AL.add)
                                else:
                                    nc.vector.tensor_tensor(
                                        out=ksum[:, c0:c0 + w_, :],
                                        in0=ksum[:, c0:c0 + w_, :],
                                        in1=kc, op=AL.add)
                                if s < 3:
                                    nc.vector.scalar_tensor_tensor(
                                        out=v[:, c0:c0 + w_, :], in0=kc,
                                        scalar=float(coef[s]),
                                        in1=x[:, c0:c0 + w_, :],
                                        op0=AL.mult, op1=AL.add)
                                else:
                                    nc.vector.scalar_tensor_tensor(
                                        out=x[:, c0:c0 + w_, :],
                                        in0=ksum[:, c0:c0 + w_, :],
                                        scalar=H_ODE / 6.0,
                                        in1=x[:, c0:c0 + w_, :],
                                        op0=AL.mult, op1=AL.add)
                                    nc.vector.tensor_copy(
                                        out=v[:, c0:c0 + w_, :],
                                        in_=x[:, c0:c0 + w_, :])
                                nc.sync.dma_start(
                                    out=sy["agin_ap"][:, c0:c0 + w_, :],
                                    in_=v[:, c0:c0 + w_, :])

                            def post_seg(si, sy=sy):
                                for ci in sy["ready"][si]:
                                    finalize_chunk(ci)

                            if "nogather" in ab:
                                for ci in range(NCH):
                                    finalize_chunk(ci)
                            else:
                                emit_seg_gathers(
                                    nc, gp, sy["table"][:, :], None,
                                    None, sy["seg"], acc, SEGW,
                                    stream=(sy["gx"], sy["m"], bsp),
                                    post_seg=post_seg,
                                    no_select=("nosel" in ab),
                                    no_gather=("selgarb" in ab))
                            if "noag" not in ab or last_sub:
                                nc.gpsimd.collective_compute(
                                    "AllGather", AL.bypass,
                                    replica_groups=RG_ALL,
                                    ins=[sy["agin"][:, :]],
                                    outs=[(sy["tableC"] if last_sub
                                           else sy["table"])[:, :]])

                # ================= PHASE C =================
                with tc.tile_pool(name="phc", bufs=2) as cpp:
                    zp = acc
                    zn = ksum_p
                    nc.vector.memset(zp[:, :, :], 0.0)
                    nc.vector.memset(zn[:, :, :], 0.0)
                    with tc.tile_pool(name="phcs", bufs=2) as csp:
                        emit_seg_gathers(nc, gp, tableCp[:, :], None, None,
                                         segC, zp, SEGW,
                                         stream=(gxCp, mCp, csp))
                        emit_seg_gathers(nc, gp, tableCn[:, :], None, None,
                                         segC, zn, SEGW,
                                         stream=(gxCn, mCn, csp))
                    zc = x_p
                    cc = 16
                    for j in range((TC + cc - 1) // cc):
                        c0 = j * cc
                        w = min(cc, TC - c0)
                        ztp = cpp.tile([128, 512], F16, tag="ztp")
                        nc.vector.transpose(
                            out=ztp[:, 0:w * 32],
                            in_=zp[:, c0:c0 + w, :]
                                .rearrange("p c f -> p (c f)"))
                        ztn = cpp.tile([128, 512], F16, tag="ztn")
                        nc.vector.transpose(
                            out=ztn[:, 0:w * 32],
                            in_=zn[:, c0:c0 + w, :]
                                .rearrange("p c f -> p (c f)"))
                        ps = pp.tile([128, cc * 32], F32, space="PSUM",
                                     tag="ps")
                        nc.tensor.matmul(
                            out=ps[:, 0:w * 32], lhsT=cwb_t[0][:, :],
                            rhs=ztp[:, 0:w * 32].rearrange(
                                "p (c f) -> p c f", f=32),
                            start=True, stop=False)
                        nc.tensor.matmul(
                            out=ps[:, 0:w * 32], lhsT=cwb_t[1][:, :],
                            rhs=ztn[:, 0:w * 32].rearrange(
                                "p (c f) -> p c f", f=32),
                            start=False, stop=True)
                        yt = cpp.tile([128, cc * 32], F32, tag="yt")
                        nc.scalar.activation(out=yt[:, 0:w * 32],
                                             in_=ps[:, 0:w * 32],
                                             func=ACTF.Identity,
                                             bias=cbstk_t[:, :], scale=1.0)
                        nc.vector.transpose(
                            out=zc[:, c0:c0 + w, :]
                                .rearrange("p c f -> p (c f)"),
                            in_=yt[:, 0:w * 32])
                    # layernorm over the 32 features
                    s1 = cpp.tile([128, TC, 1], F32, tag="s1")
                    nc.vector.tensor_reduce(out=s1[:, :, :],
                                            in_=zc[:, :, :],
                                            axis=mybir.AxisListType.X,
                                            op=AL.add)
                    nc.vector.tensor_scalar_mul(s1[:, :, :], s1[:, :, :],
                                                -1.0 / 32.0)
                    nc.vector.tensor_tensor(
                        out=zc[:, :, :], in0=zc[:, :, :],
                        in1=s1[:, :, :].to_broadcast([128, TC, 32]),
                        op=AL.add)
                    sq = x_n
                    nc.scalar.activation(out=sq[:, :, :], in_=zc[:, :, :],
                                         func=ACTF.Square)
                    v1 = cpp.tile([128, TC, 1], F32, tag="v1")
                    nc.vector.tensor_reduce(out=v1[:, :, :],
                                            in_=sq[:, :, :],
                                            axis=mybir.AxisListType.X,
                                            op=AL.add)
                    nc.vector.tensor_scalar(
                        out=v1[:, :, :], in0=v1[:, :, :],
                        scalar1=1.0 / 32.0,
                        scalar2=1e-5, op0=AL.mult, op1=AL.add)
                    nc.scalar.activation(out=v1[:, :, :], in_=v1[:, :, :],
                                         func=ACTF.Sqrt)
                    nc.vector.reciprocal(v1[:, :, :], v1[:, :, :])
                    nc.vector.tensor_tensor(
                        out=zc[:, :, :], in0=zc[:, :, :],
                        in1=v1[:, :, :].to_broadcast([128, TC, 32]),
                        op=AL.mult)
                    nc.vector.tensor_tensor(
                        out=zc[:, :, :], in0=zc[:, :, :],
                        in1=lng_t[:, :, :].to_broadcast([128, TC, 32]),
                        op=AL.mult)
                    nc.vector.tensor_tensor(
                        out=zc[:, :, :], in0=zc[:, :, :],
                        in1=lnb_t[:, :, :].to_broadcast([128, TC, 32]),
                        op=AL.add)
                    nc.sync.dma_start(out=pos_packed_dram_ap(out_o, TC, 32),
                                      in_=zc[:, :, :])
    nc.compile()
    return nc


# ============================ entry point ============================
_CACHE = {}


def kernel(**inputs):
    from concourse.bass_utils import run_bass_kernel_spmd

    cfg, in_maps, meta = build_all(inputs)
    key = "prog"
    if key not in _CACHE:
        _CACHE[key] = build_program(cfg)
    nc = _CACHE[key]
    br = run_bass_kernel_spmd(nc, in_maps, list(range(8)))
    return assemble_output(br.results, meta["perA"],
                           inputs["H_t"].shape[0]).astype(np.float32)
